# revision 1
# baseline (speedup 1.0000x reference)
"""Multi-head attention kernel for Trainium2, 8 NeuronCores.

Sharding: core c -> (batch b = c // 4, head group g = c % 4 covering heads
g*4 .. g*4+3).  Each core computes its heads' Q/K/V projections, biased
masked softmax attention, and a partial output projection through its rows
of Wo.  The host sums the 4 partial outputs per batch (the Wo all-reduce)
and adds bo.

Device layout choice: scores are computed TRANSPOSED ([k, q]) so the
post-exp attention tiles feed the attn@V matmul directly as the moving
operand -- no on-device transposes anywhere.  To support this, the host
feeds q/k/v transposed ([HID, S]) and attn_bias transposed per head
([S_k, S_q]); that is purely a shard-layout choice done during input prep.

Mask handling: additive -1e9 key-mask row enters the scores matmul as a
65th contraction row (ones row in qhT, maskadd row in khT).  Softmax skips
max-subtraction (scores ~ N(0,2); exp cannot overflow fp32).  Row-sums fall
out of the attn@V matmul as a 65th output row via a ones column in V.
Query-row masking commutes with the output projection and is applied as a
per-partition scale on the final tiles.
"""

import numpy as np

B, S, HID, H, DK = 2, 2048, 1024, 16, 64
SCALE = DK ** -0.5
NEG = -1000000000.0
NCORES = 8
GROUPS = NCORES // B      # 4 cores per batch
HPC = H // GROUPS         # 4 heads per core
QCH = 4                   # q chunks of 512
KT = 16                   # k tiles of 128
ST = 4                    # s-tiles (128 rows) per q chunk
NH = 512                  # matmul moving free dim / PSUM bank

_NC_CACHE = {}


def _build_nc():
    import concourse.tile as tile
    from concourse import bacc, mybir

    f32 = mybir.dt.float32
    AF = mybir.ActivationFunctionType
    OP = mybir.AluOpType

    nc = bacc.Bacc(None, target_bir_lowering=False, debug=True)

    qT = nc.dram_tensor("qT", [HID, S], f32, kind="ExternalInput")
    kT = nc.dram_tensor("kT", [HID, S], f32, kind="ExternalInput")
    vT = nc.dram_tensor("vT", [HID, S], f32, kind="ExternalInput")
    biasT = nc.dram_tensor("biasT", [HPC, S, S], f32, kind="ExternalInput")
    maskadd = nc.dram_tensor("maskadd", [1, S], f32, kind="ExternalInput")
    maskq = nc.dram_tensor("maskq", [1, S], f32, kind="ExternalInput")
    Wq_l = nc.dram_tensor("Wq_l", [HID, HPC * DK], f32, kind="ExternalInput")
    Wk_l = nc.dram_tensor("Wk_l", [HID, HPC * DK], f32, kind="ExternalInput")
    Wv_l = nc.dram_tensor("Wv_l", [HID, HPC * DK], f32, kind="ExternalInput")
    Wo_l = nc.dram_tensor("Wo_l", [HPC * DK, H * DK], f32, kind="ExternalInput")
    bq_l = nc.dram_tensor("bq_l", [HPC * DK], f32, kind="ExternalInput")  # pre-scaled
    bk_l = nc.dram_tensor("bk_l", [HPC * DK], f32, kind="ExternalInput")
    bv_l = nc.dram_tensor("bv_l", [HPC * DK], f32, kind="ExternalInput")
    out_d = nc.dram_tensor("out", [S, H * DK], f32, kind="ExternalOutput")

    with tile.TileContext(nc) as tc:
        with tc.tile_pool(name="const", bufs=1) as const, \
             tc.tile_pool(name="persist", bufs=1) as persist:
            # small constants
            Wo_sb = const.tile([DK, HPC, H * DK], f32, name="Wo_sb", tag="Wo_sb")
            nc.sync.dma_start(out=Wo_sb[:], in_=Wo_l.rearrange("(h p) o -> p h o", p=DK))
            bq_sb = const.tile([DK, HPC], f32, name="bq_sb", tag="bq_sb")
            nc.sync.dma_start(out=bq_sb[:], in_=bq_l.rearrange("(h p) -> p h", p=DK))
            bk_sb = const.tile([DK, HPC], f32, name="bk_sb", tag="bk_sb")
            nc.sync.dma_start(out=bk_sb[:], in_=bk_l.rearrange("(h p) -> p h", p=DK))
            bv_row = const.tile([1, HPC * DK], f32, name="bv_row", tag="bv_row")
            nc.sync.dma_start(out=bv_row[:], in_=bv_l.rearrange("(o d) -> o d", o=1))
            ones1 = const.tile([1, 128], f32, name="ones1", tag="ones1")
            nc.vector.memset(ones1[:], 1.0)
            ones64 = const.tile([65, DK], f32, name="ones64", tag="ones64")
            nc.vector.memset(ones64[64:65, :], 1.0)
            # query mask as per-s-tile per-partition columns [128, 16]
            mq_sb = const.tile([128, S // 128], f32, name="mq_sb", tag="mq_sb")
            nc.sync.dma_start(out=mq_sb[:], in_=maskq.rearrange("o (t p) -> p (o t)", p=128))

            # persistent per-head activations
            qhT = []
            khT = []
            for h in range(HPC):
                t_q = persist.tile([65, S], f32, name=f"qhT{h}", tag=f"qhT{h}")
                nc.vector.memset(t_q[64:65, :], 1.0)  # ones row for maskadd contraction
                qhT.append(t_q)
                t_k = persist.tile([65, S], f32, name=f"khT{h}", tag=f"khT{h}")
                nc.sync.dma_start(out=t_k[64:65, :], in_=maskadd[:])
                khT.append(t_k)
            vh_sb = persist.tile([128, KT, HPC, DK + 1], f32, name="vh_sb", tag="vh_sb")
            nc.vector.memset(vh_sb[:, :, :, DK:DK + 1], 1.0)  # ones col -> rowsums

            # ---- projection phase ----
            with tc.tile_pool(name="proj_act", bufs=1) as actp, \
                 tc.tile_pool(name="proj_w", bufs=2) as wp, \
                 tc.tile_pool(name="proj_ps", bufs=4, space="PSUM") as pps, \
                 tc.tile_pool(name="proj_psv", bufs=2, space="PSUM") as ppsv:

                def qk_proj(src_dram, W_dram, dst_tiles, bias_sb, scale):
                    W_sb = wp.tile([128, HID // 128, HPC * DK], f32, name="W_sb", tag="W")
                    nc.sync.dma_start(out=W_sb[:], in_=W_dram.rearrange("(t p) d -> p t d", p=128))
                    a_sb = actp.tile([128, HID // 128, S], f32, name="a_sb", tag="act")
                    nc.sync.dma_start(out=a_sb[:], in_=src_dram.rearrange("(t p) s -> p t s", p=128))
                    for h in range(HPC):
                        for sc in range(QCH):
                            ps = pps.tile([DK, NH], f32, name="ps_qk", tag="ps")
                            for t in range(HID // 128):
                                nc.tensor.matmul(
                                    ps[:],
                                    lhsT=W_sb[:, t, h * DK:(h + 1) * DK],
                                    rhs=a_sb[:, t, sc * NH:(sc + 1) * NH],
                                    start=(t == 0), stop=(t == HID // 128 - 1),
                                )
                            nc.scalar.activation(
                                out=dst_tiles[h][0:DK, sc * NH:(sc + 1) * NH],
                                in_=ps[:], func=AF.Identity,
                                bias=bias_sb[:, h:h + 1], scale=scale,
                            )

                qk_proj(qT, Wq_l, qhT, bq_sb, float(SCALE))
                qk_proj(kT, Wk_l, khT, bk_sb, 1.0)

                # v projection: vh[k, d] natural layout (k on partitions)
                Wv_sb = wp.tile([128, HID // 128, HPC * DK], f32, name="Wv_sb", tag="W")
                nc.sync.dma_start(out=Wv_sb[:], in_=Wv_l.rearrange("(t p) d -> p t d", p=128))
                v_sb = actp.tile([128, HID // 128, S], f32, name="v_sb", tag="act")
                nc.sync.dma_start(out=v_sb[:], in_=vT.rearrange("(t p) s -> p t s", p=128))
                for kt in range(KT):
                    psv = ppsv.tile([128, HPC * DK], f32, name="psv", tag="psv")
                    for t in range(HID // 128):
                        nc.tensor.matmul(
                            psv[:],
                            lhsT=v_sb[:, t, kt * 128:(kt + 1) * 128],
                            rhs=Wv_sb[:, t, :],
                            start=(t == 0), stop=False,
                        )
                    nc.tensor.matmul(psv[:], lhsT=ones1[:], rhs=bv_row[:],
                                     start=False, stop=True)
                    nc.scalar.activation(
                        out=vh_sb[:, kt, :, 0:DK],
                        in_=psv.rearrange("p (h d) -> p h d", h=HPC),
                        func=AF.Copy,
                    )

            # ---- main attention loop ----
            with tc.tile_pool(name="bias_p", bufs=2) as bp, \
                 tc.tile_pool(name="xn_p", bufs=2) as xp, \
                 tc.tile_pool(name="small_p", bufs=2) as sp, \
                 tc.tile_pool(name="out_p", bufs=2) as outp, \
                 tc.tile_pool(name="ps_sc", bufs=3, space="PSUM") as scp, \
                 tc.tile_pool(name="ps_x", bufs=2, space="PSUM") as xps, \
                 tc.tile_pool(name="ps_bc", bufs=1, space="PSUM") as bcp, \
                 tc.tile_pool(name="ps_o", bufs=2, space="PSUM") as ops:
                for qc in range(QCH):
                    qsl = slice(qc * NH, (qc + 1) * NH)
                    xn = [xp.tile([DK, NH], f32, name=f"xn{h}", tag=f"xn{h}")
                          for h in range(HPC)]
                    for h in range(HPC):
                        bt = bp.tile([128, KT, NH], f32, name="bt", tag="bt")
                        nc.sync.dma_start(
                            out=bt[:],
                            in_=biasT[h].rearrange("(kt p) q -> p kt q", p=128)[:, :, qsl],
                        )
                        # scores^T tiles [128k, 512q], add bias in SBUF
                        for kt in range(KT):
                            sc_ps = scp.tile([128, NH], f32, name="sc_ps", tag="sc")
                            nc.tensor.matmul(
                                sc_ps[:],
                                lhsT=khT[h][:, kt * 128:(kt + 1) * 128],
                                rhs=qhT[h][:, qsl],
                                start=True, stop=True,
                            )
                            nc.vector.scalar_tensor_tensor(
                                out=bt[:, kt, :], in0=sc_ps[:], scalar=0.0,
                                in1=bt[:, kt, :], op0=OP.add, op1=OP.add,
                            )
                        nc.scalar.activation(out=bt[:], in_=bt[:], func=AF.Exp)
                        # attn @ V (+ rowsum via ones column), accumulate over k
                        xt = xps.tile([DK + 1, NH], f32, name="xt", tag="xt")
                        for kt in range(KT):
                            nc.tensor.matmul(
                                xt[:],
                                lhsT=vh_sb[:, kt, h, :],
                                rhs=bt[:, kt, :],
                                start=(kt == 0), stop=(kt == KT - 1),
                            )
                        # normalize: xn = xt[0:64] * broadcast(1 / rowsum)
                        rs = sp.tile([65, NH], f32, name="rs", tag="rs")
                        nc.scalar.activation(out=rs[64:65, :], in_=xt[DK:DK + 1, :],
                                             func=AF.Copy)
                        nc.vector.reciprocal(out=rs[64:65, :], in_=rs[64:65, :])
                        bc = bcp.tile([DK, NH], f32, name="bc", tag="bc")
                        nc.tensor.matmul(bc[:], lhsT=ones64[64:65, :],
                                         rhs=rs[64:65, :], start=True, stop=True)
                        xe = sp.tile([DK, NH], f32, name="xe", tag="xe")
                        nc.scalar.activation(out=xe[:], in_=xt[0:DK, :], func=AF.Copy)
                        nc.vector.tensor_mul(xn[h][:], xe[:], bc[:])
                    # partial output projection for this q chunk
                    for st in range(ST):
                        stg = qc * ST + st
                        ot = outp.tile([128, H * DK], f32, name="ot", tag="ot")
                        for oh in range(2):
                            po = ops.tile([128, NH], f32, name="po", tag="po")
                            for h in range(HPC):
                                nc.tensor.matmul(
                                    po[:],
                                    lhsT=xn[h][:, st * 128:(st + 1) * 128],
                                    rhs=Wo_sb[:, h, oh * NH:(oh + 1) * NH],
                                    start=(h == 0), stop=(h == HPC - 1),
                                )
                            # apply query-row mask as per-partition scale
                            nc.scalar.activation(
                                out=ot[:, oh * NH:(oh + 1) * NH], in_=po[:],
                                func=AF.Copy, scale=mq_sb[:, stg:stg + 1],
                            )
                        nc.sync.dma_start(
                            out=out_d[stg * 128:(stg + 1) * 128, :], in_=ot[:],
                        )

    nc.compile()
    return nc


def _get_nc():
    if "nc" not in _NC_CACHE:
        _NC_CACHE["nc"] = _build_nc()
    return _NC_CACHE["nc"]


def kernel(q, k, v, attn_bias, mask, Wq, bq, Wk, bk, Wv, bv, Wo, bo):
    from concourse.bass_utils import run_bass_kernel_spmd

    nc = _get_nc()

    q = np.asarray(q, np.float32)
    k = np.asarray(k, np.float32)
    v = np.asarray(v, np.float32)
    attn_bias = np.asarray(attn_bias, np.float32)
    mask_i = np.asarray(mask)
    Wq = np.asarray(Wq, np.float32); bq = np.asarray(bq, np.float32)
    Wk = np.asarray(Wk, np.float32); bk = np.asarray(bk, np.float32)
    Wv = np.asarray(Wv, np.float32); bv = np.asarray(bv, np.float32)
    Wo = np.asarray(Wo, np.float32); bo = np.asarray(bo, np.float32)

    qTb = [np.ascontiguousarray(q[b].T) for b in range(B)]
    kTb = [np.ascontiguousarray(k[b].T) for b in range(B)]
    vTb = [np.ascontiguousarray(v[b].T) for b in range(B)]
    maskadd_b = [
        np.where(mask_i[b] == 0, np.float32(NEG), np.float32(0.0))[None, :]
        .astype(np.float32) for b in range(B)
    ]
    maskq_b = [mask_i[b].astype(np.float32)[None, :] for b in range(B)]

    in_maps = []
    for c in range(NCORES):
        b = c // GROUPS
        g = c % GROUPS
        hsl = slice(g * HPC * DK, (g + 1) * HPC * DK)
        in_maps.append({
            "qT": qTb[b],
            "kT": kTb[b],
            "vT": vTb[b],
            "biasT": np.ascontiguousarray(
                attn_bias[b, g * HPC:(g + 1) * HPC].transpose(0, 2, 1)),
            "maskadd": maskadd_b[b],
            "maskq": maskq_b[b],
            "Wq_l": np.ascontiguousarray(Wq[:, hsl]),
            "Wk_l": np.ascontiguousarray(Wk[:, hsl]),
            "Wv_l": np.ascontiguousarray(Wv[:, hsl]),
            "Wo_l": np.ascontiguousarray(Wo[hsl, :]),
            "bq_l": np.ascontiguousarray(bq[hsl]) * np.float32(SCALE),
            "bk_l": np.ascontiguousarray(bk[hsl]),
            "bv_l": np.ascontiguousarray(bv[hsl]),
        })

    res = run_bass_kernel_spmd(nc, in_maps, core_ids=list(range(NCORES)))

    out = np.zeros((B, S, H * DK), np.float32)
    for c in range(NCORES):
        out[c // GROUPS] += res.results[c]["out"]
    out += bo[None, None, :]
    return out
